# revision 31
# baseline (speedup 1.0000x reference)
"""ArcFace margin loss (ArcMarginLoss) on 8 Trainium2 NeuronCores.

Classification-parallel sharding: V=32000 classes split across 8 cores
(4000 each, padded to 4096).  The device kernel is a pure fp8 GEMM +
exp-rowsum pipeline; everything O(B*D) or O(V*D)-elementwise lives on the
host:

Host prep (numpy):
  - x-hat = x/|x|, w-hat = W/|W| rows (fp32), scaled by 16 and cast to
    fp8 e4m3.  PSUM then holds 256*cos, and the scalar-engine exp applies
    scale s/256 and bias -s, computing exp(s*cos - 30) directly: since
    cos <= 1 no global max pass or cross-core collective is needed.
  - Both operands are packed K-major (contraction dim on partitions) in
    the exact SBUF layout, as (j, i) pairs for the PE's fp8 DoubleRow
    mode (two 128-deep k-planes per pass, 2x MAC throughput).
  - The label-column path (cos_y, phi = cos(theta+m), the per-row
    exp corrections) is O(B*D) and computed on the host in fp32.

Device per core (one SPMD NEFF):
  - DMA xT [128, 16m, 2j, 2i, 128] and wT [128, 2j, 2i, 4096] fp8.
  - 16 m-tiles x 2 chunks: 8 DoubleRow matmuls fill a [128, 2048] fp32
    PSUM tile (4 banks, double-buffered), then one scalar-engine
    activation computes exp(s*cos - 30) with a fused row-sum
    (accum_out).  The 96 pad classes are simply never computed (the
    c1 chunk is trimmed to 1952 real columns).
  - Output: per-row partial sums S_c [2048] fp32 (p-major).

Host epilogue: S = sum_c S_c, scatter-add the label corrections,
loss = mean(30 + log(S) - s*phi_label).
"""

import math
import numpy as np
from contextlib import ExitStack

import concourse.bass as bass
import concourse.tile as tile
from concourse import bacc, mybir
from concourse import bass_utils
from concourse._compat import with_exitstack
from concourse.masks import make_identity

P = 128
B = 2048          # batch rows
D = 512           # feature dim
V = 32000         # classes
NCORES = 8
VS = V // NCORES  # 4000 classes per core
VSP = 4096        # padded shard size
MT = B // P       # 16 batch row tiles
NJ = 2            # DoubleRow passes over D (each contracts 256)
CHUNK = 1024      # psum chunk width (2 banks; 4 PSUM buffers in flight)
NQ = 4            # class quarters per m-tile
QW = (1024, 1024, 1024, 928)  # real columns per quarter (q3 trims the pad)

S_SCALE = 30.0
M_MARGIN = 0.5
SHIFT = 30.0      # exp(logit - SHIFT): logits <= 30 so always <= 0
WS = 16.0         # fp8 encode scale for x-hat and w-hat
EPS = 1e-12

F32 = mybir.dt.float32
BF16 = mybir.dt.bfloat16
F8 = mybir.dt.float8e4
I32 = mybir.dt.int32
OP = mybir.AluOpType
AF = mybir.ActivationFunctionType
AX = mybir.AxisListType
DR = mybir.MatmulPerfMode.DoubleRow

# Schraudolph exp-via-int-bits constants for the DVE-offloaded chunks:
# i32 = A*psum + B, bitcast float ~= exp(escale*psum - SHIFT).  C recenters
# the 2^frac-vs-exp sawtooth so the MEAN multiplicative error over the
# logit distribution is ~1 (sums are unbiased; residual is ~2% noise per
# element that averages out across a 2048-class chunk).
_LOG2E = 1.4426950408889634
_SCHR_C = -482247.389
A_DVE = float(np.float32((S_SCALE / (WS * WS)) * (2.0 ** 23) * _LOG2E))
B_DVE = float(np.float32(127.0 * 2 ** 23 - SHIFT * (2 ** 23) * _LOG2E + _SCHR_C))
# chunks (k = q*16 + m) whose exp+sum runs on the DVE instead of the
# scalar engine, keeping the scalar engine's total drain time under the
# PE fill time.  20 of 64 chunks, min spacing 3.
DVE_CHUNKS = frozenset(k for k in range(64) if k % 16 in (0, 3, 6, 9, 12))


@with_exitstack
def _arc_kernel(ctx: ExitStack, tc: tile.TileContext,
                xt_d: bass.AP, wt_d: bass.AP, s_d: bass.AP):
    nc = tc.nc

    sb = ctx.enter_context(tc.tile_pool(name="sb", bufs=1))
    exs = ctx.enter_context(tc.tile_pool(name="exs", bufs=1))
    dvs = ctx.enter_context(tc.tile_pool(name="dvs", bufs=1))
    ps = ctx.enter_context(tc.tile_pool(name="ps", bufs=4, space="PSUM"))

    xT = sb.tile([P, MT, NJ, 2, P], F8)     # [p, m, j, i, c]
    wT = sb.tile([P, NJ, 2, VSP], F8)       # [p, j, i, v]
    Spart = sb.tile([P, NQ, MT], F32)       # per-chunk row sums
    ident = sb.tile([P, P], BF16)
    make_identity(nc, ident)

    nbias = sb.tile([P, 1], F32)            # -SHIFT bias for the exp
    nc.vector.memset(nbias, -SHIFT)
    zt = sb.tile([P, 1], F32)
    nc.vector.memset(zt, 0.0)
    e0 = sb.tile([P, 1], F32)

    # DMA order: x slab 0, then quarter-0 weights in 512-col pieces (the
    # first chunk's j1 matmuls are the critical prefix), then the rest.
    nc.gpsimd.dma_start(out=xT[:, 0:4], in_=xt_d[:, 0:4])
    nc.gpsimd.dma_start(out=wT[:, 0, :, 0:512], in_=wt_d[:, 0, :, 0:512])
    nc.gpsimd.dma_start(out=wT[:, 1, :, 0:512], in_=wt_d[:, 1, :, 0:512])
    nc.gpsimd.dma_start(out=wT[:, 0, :, 512:CHUNK], in_=wt_d[:, 0, :, 512:CHUNK])
    nc.gpsimd.dma_start(out=wT[:, 1, :, 512:CHUNK], in_=wt_d[:, 1, :, 512:CHUNK])
    for s in range(1, 4):
        nc.gpsimd.dma_start(out=xT[:, 4 * s:4 * s + 4], in_=xt_d[:, 4 * s:4 * s + 4])
    for q in range(1, NQ):
        lo, hi = q * CHUNK, (q + 1) * CHUNK
        nc.gpsimd.dma_start(out=wT[:, 0, :, lo:hi], in_=wt_d[:, 0, :, lo:hi])
        nc.gpsimd.dma_start(out=wT[:, 1, :, lo:hi], in_=wt_d[:, 1, :, lo:hi])

    # Load the Exp table during the DMA prefix (1.3us once).
    nc.scalar.activation(out=e0, in_=zt, func=AF.Exp, bias=nbias)

    # PE warm-up: dependency-free transposes ramp the PE p-state to 2.4GHz
    # while the prefix DMAs land.
    for _ in range(22):
        wtile = ps.tile([P, CHUNK], F32, tag="mm", name="warm")
        nc.tensor.transpose(wtile[:, 0:64].bitcast(BF16), ident, ident)

    escale = float(S_SCALE / (WS * WS))
    for q in range(NQ):
        qbase = q * CHUNK
        w = QW[q]
        for m in range(MT):
            pm = ps.tile([P, CHUNK], F32, tag="mm", name="pm")
            for j in range(NJ):
                for b in range(2):
                    lo = b * 512
                    hi = min(lo + 512, w)
                    nc.tensor.matmul(
                        pm[:, lo:hi],
                        xT[:, m, j],
                        wT[:, j, :, qbase + lo:qbase + hi],
                        start=(j == 0), stop=(j == NJ - 1),
                        perf_mode=DR)
            k = q * MT + m
            if k in DVE_CHUNKS:
                yi = dvs.tile([P, CHUNK], I32, tag="yi", name="yi")
                nc.vector.tensor_scalar(yi[:, :w], pm[:, :w], A_DVE, B_DVE,
                                        OP.mult, OP.add)
                nc.vector.tensor_reduce(
                    out=Spart[:, q, m:m + 1], in_=yi[:, :w].bitcast(F32),
                    axis=AX.X, op=OP.add)
            else:
                ex = exs.tile([P, CHUNK], BF16, tag="ex", name="ex")
                nc.scalar.activation(
                    out=ex[:, :w], in_=pm[:, :w], func=AF.Exp,
                    bias=nbias, scale=escale,
                    accum_out=Spart[:, q, m:m + 1])

    # DMA the per-chunk sums directly; host adds the quarters.
    nc.sync.dma_start(out=s_d.rearrange("(p q m) -> p q m", p=P, q=NQ),
                      in_=Spart)


def build_bass():
    nc = bacc.Bacc("TRN2", target_bir_lowering=False, debug=False,
                   enable_asserts=False, num_devices=NCORES)
    xt_d = nc.dram_tensor("xt_in", [P, MT, NJ, 2, P], F8,
                          kind="ExternalInput").ap()
    wt_d = nc.dram_tensor("wt_in", [P, NJ, 2, VSP], F8,
                          kind="ExternalInput").ap()
    s_d = nc.dram_tensor("s_out", [NQ * B], F32, kind="ExternalOutput").ap()
    with tile.TileContext(nc) as tc:
        _arc_kernel(tc, xt_d, wt_d, s_d)
    nc.compile()
    return nc


_NC = None


def _get_nc():
    global _NC
    if _NC is None:
        _NC = build_bass()
    return _NC


def _pm(vec, nt):
    """host-side inverse of the device's p-major [(p, m)] output layout."""
    return vec.reshape(P, nt).T.reshape(-1)


def make_in_maps(xn: np.ndarray, W: np.ndarray):
    import ml_dtypes
    F8NP = ml_dtypes.float8_e4m3

    xq = (xn * WS).astype(F8NP)                      # [B, D]
    # xt[p, m, j, i, c] = xq[m*128 + c, j*256 + i*128 + p]
    xt = np.ascontiguousarray(
        xq.reshape(MT, P, NJ, 2, P).transpose(4, 0, 2, 3, 1))

    wnorm = np.linalg.norm(W, axis=1, keepdims=True)
    Wn = W / np.maximum(wnorm, EPS)
    in_maps = []
    for c in range(NCORES):
        wq = np.zeros((VSP, D), dtype=F8NP)
        wq[:VS] = (Wn[c * VS:(c + 1) * VS] * WS).astype(F8NP)
        # wt[p, j, i, v] = wq[v, j*256 + i*128 + p]
        wt = np.ascontiguousarray(
            wq.reshape(VSP, NJ, 2, P).transpose(3, 1, 2, 0))
        in_maps.append({"xt_in": xt, "wt_in": wt})
    return in_maps, Wn


def kernel(x, W, labels, **run_kwargs):
    x = np.ascontiguousarray(np.asarray(x), dtype=np.float32)
    W = np.ascontiguousarray(np.asarray(W), dtype=np.float32)
    lab = np.asarray(labels).astype(np.int64)
    assert x.shape == (B, D) and W.shape == (V, D) and lab.shape == (B,), \
        (x.shape, W.shape, lab.shape)

    xn = x / np.maximum(np.linalg.norm(x, axis=1, keepdims=True), EPS)

    nc = _get_nc()
    in_maps, Wn = make_in_maps(xn, W)
    res = bass_utils.run_bass_kernel_spmd(
        nc, in_maps, core_ids=list(range(NCORES)), **run_kwargs)

    S = np.zeros(B, dtype=np.float64)
    for r in res.results:
        sp = r["s_out"].reshape(P, NQ, MT).sum(axis=1)  # add the quarters
        S += sp.T.reshape(-1).astype(np.float64)

    # Host label-column correction (O(B*D), fp64 epilogue).
    cos_y = np.einsum("bd,bd->b", xn.astype(np.float64),
                      Wn[lab].astype(np.float64))
    sin_y = np.sqrt(np.clip(1.0 - cos_y * cos_y, 0.0, 1.0))
    phi_y = cos_y * math.cos(M_MARGIN) - sin_y * math.sin(M_MARGIN)
    S += np.exp(S_SCALE * phi_y - SHIFT) - np.exp(S_SCALE * cos_y - SHIFT)
    loss = np.mean(SHIFT + np.log(S) - S_SCALE * phi_y)

    kernel.last_results = res
    return np.asarray(loss, dtype=np.float32)
